# revision 8
# baseline (speedup 1.0000x reference)
"""Trainium2 Bass kernel for EuclideanSimilarity (retrieval_knn).

Reference computation per batch b (B=8, L=4096, D=128):
    projected = x @ W.T + b                      [L, D]
    q = avgpool2(x) @ W.T + b                    [L/2, D]   (== avgpool2(projected))
    power = ||q_i||^2 + ||k_j||^2 - 2 q_i.k_j    [L/2, L]
    sim = exp(-sqrt(max(power, 0)))
    k = sim @ projected                          [L/2, D]
    returns (q, k, v=k)

Sharding: data-parallel over batch, one batch element per NeuronCore (8 cores).

v2 design notes (per core):
  - The ACT (scalar) engine is the roofline: sqrt + exp over the full
    [2048, 4096] sim matrix = 2 x 65536 elem/lane at ~0.72 ns = ~94 us.
    Everything else is arranged to (a) never add work to ACT and (b) cut
    ACT table-set switches to the minimum 2 (one sqrt batch, one exp
    batch over the WHOLE matrix, held as fp16 strips: 4 chunks x
    [128, 16384] fp16 = 128 KB/partition).
  - All GEMM2/GEMM3 operands are fp16 (1 cyc/row on PE, same as f32r,
    half the SBUF); phase-1 GEMMs are f32r (1 cyc/row at >=256 moving
    cols). Verified vs fp64 reference: k rel err ~3.7e-4.
  - power = (-2qk psum) + qsq[i] + ksq[j] in ONE fused DVE op writing the
    fp16 strip; sqrt and exp run in-place on the strip; GEMM3 reads it.
"""

import os
import sys

for _p in ("/opt/trn_rl_repo", "/root/.axon_site/_ro/trn_rl_repo"):
    if os.path.isdir(_p) and _p not in sys.path:
        sys.path.insert(0, _p)

import numpy as np

import concourse.bass as bass
import concourse.mybir as mybir
from concourse import bacc
from concourse.bass_utils import run_bass_kernel_spmd
from concourse.tile import TileContext
from concourse.tile_rust import add_dep_helper

B, L, D = 8, 4096, 128
LQ = L // 2          # 2048 pooled queries
P = 128              # partitions
NI = 512             # i-chunk (queries per chunk)
NCHUNK = LQ // NI    # 4
NJT = L // P         # 32 j-tiles
NQ = 8               # ACT sub-strips per chunk
F32 = mybir.dt.float32
F32R = mybir.dt.float32r
F16 = mybir.dt.float16

AF = mybir.ActivationFunctionType
ALU = mybir.AluOpType


def build_nc(repeat=1):
    nc = bacc.Bacc("TRN2", target_bir_lowering=False)

    xT = nc.declare_dram_parameter("xT", [P, L], F32R, isOutput=False)
    WT = nc.declare_dram_parameter("WT", [P, D], F32R, isOutput=False)       # W.T
    Wm2T = nc.declare_dram_parameter("Wm2T", [P, D], F32R, isOutput=False)   # (-2W).T
    bcols = nc.declare_dram_parameter("bcols", [P, 2], F32, isOutput=False)  # [b, -2b]
    b_bcast_in = nc.declare_dram_parameter("b_bcast", [P, D], F32, isOutput=False)
    ones_in = nc.declare_dram_parameter("ones_mat", [P, P], F32R, isOutput=False)

    qT_out = nc.declare_dram_parameter("qT", [P, LQ], F32, isOutput=True)
    kT_out = nc.declare_dram_parameter("kT", [P, LQ], F32, isOutput=True)

    with TileContext(nc) as tc:
      for _rep in range(repeat):
        with (
            tc.tile_pool(name="consts", bufs=1) as consts,
            tc.tile_pool(name="big", bufs=1) as big,
            tc.tile_pool(name="work", bufs=4) as work,
            tc.tile_pool(name="ps1", bufs=4, space="PSUM") as ps1,
        ):
            # ---- constants ----
            WT_sb = consts.tile([P, D], F32R)
            Wm2T_sb = consts.tile([P, D], F32R)
            bcols_sb = consts.tile([P, 2], F32)
            b_bcast = consts.tile([P, D], F32)
            ones_sb = consts.tile([P, P], F32R)
            nc.sync.dma_start(out=WT_sb[:], in_=WT[:])
            nc.sync.dma_start(out=Wm2T_sb[:], in_=Wm2T[:])
            nc.sync.dma_start(out=bcols_sb[:], in_=bcols[:])
            nc.sync.dma_start(out=b_bcast[:], in_=b_bcast_in[:])
            nc.sync.dma_start(out=ones_sb[:], in_=ones_in[:])
            b_col = bcols_sb[:, 0:1]
            bm2_col = bcols_sb[:, 1:2]

            projTm2 = big.tile([P, L], F16)     # GEMM2 stationary (-2 proj)^T
            projnat = big.tile([P, L], F16)     # GEMM3 stationary [l, e] tiles
            qT_mm = big.tile([P, LQ], F16)      # GEMM2 moving (q^T)
            qs16 = big.tile([P, LQ], F16)       # qsq broadcast over partitions
            ksq = consts.tile([P, NJT], F32)

            act_chain = {"i": None}

            def act(out_ap, in_ap, func, **kw):
                s = nc.scalar.activation(out_ap, in_ap, func, **kw)
                if act_chain["i"] is not None:
                    add_dep_helper(s.ins, act_chain["i"].ins, sync=False,
                                   reason="act table batch order")
                act_chain["i"] = s
                return s

            # ---- phase 1 (xT-dependent); pool closed before strips open ----
            with tc.tile_pool(name="phase1", bufs=1) as ph1:
                xT_sb = ph1.tile([P, L], F32R)
                qT_sb = ph1.tile([P, LQ], F32, tag="qT_sb", name="qT_sb")
                for c in range(L // 512):
                    nc.sync.dma_start(
                        out=xT_sb[:, c * 512:(c + 1) * 512],
                        in_=xT[:, c * 512:(c + 1) * 512])

                # projT_m2[e, l] = -2 (W x + b)^T via f32r GEMM; qT pooled
                # off the fp32 PSUM (pooling commutes with the projection).
                for c in range(L // 512):
                    ps = ps1.tile([P, 512], F32, tag="ps1")
                    nc.tensor.matmul(
                        ps, Wm2T_sb[:], xT_sb[:, c * 512:(c + 1) * 512],
                        start=True, stop=True,
                    )
                    seg = projTm2[:, c * 512:(c + 1) * 512]
                    nc.vector.tensor_scalar_add(seg, ps, bm2_col)
                    # q = -0.25 * (projTm2[2i] + projTm2[2i+1]); the -2b
                    # baked into projTm2 becomes +b after the -0.25 scale.
                    # Runs on gpsimd (SBUF-only) to keep DVE free.
                    sp = seg.rearrange("p (i two) -> p i two", two=2)
                    qtmp = work.tile([P, 256], F32, tag="qtmp")
                    nc.gpsimd.tensor_add(qtmp[:], sp[:, :, 0], sp[:, :, 1])
                    nc.gpsimd.tensor_scalar_mul(
                        qT_sb[:, c * 256:(c + 1) * 256], qtmp[:], -0.25)
                nc.sync.dma_start(out=qT_out[:], in_=qT_sb[:])
                nc.gpsimd.tensor_copy(qT_mm[:], qT_sb[:])

                # proj_nat tiles [l(128), e] fp16 and fp32 ksq
                for t in range(NJT):
                    ps = ps1.tile([P, D], F32, tag="ps1")
                    nc.tensor.matmul(
                        ps, xT_sb[:, t * P:(t + 1) * P], WT_sb[:],
                        start=True, stop=True,
                    )
                    seg16 = projnat[:, t * P:(t + 1) * P]
                    nc.vector.tensor_add(seg16, ps, b_bcast[:])
                    # ksq[:, t] = sum_e seg^2 in one ACT op (Square is in
                    # the sqrt table set; ACT is idle during phase 1)
                    sq = work.tile([P, D], F32, tag="sqs")
                    act(sq[:], seg16, AF.Square, accum_out=ksq[:, t:t + 1])

                # qs16[p, i] = ||q_i||^2 broadcast to all partitions:
                # all-ones stationary does reduce + broadcast in one matmul.
                sq_qT = ph1.tile([P, LQ], F32R)
                nc.gpsimd.tensor_mul(sq_qT[:], qT_sb[:], qT_sb[:])
                for c in range(LQ // 512):
                    ps = ps1.tile([P, 512], F32, tag="ps1")
                    nc.tensor.matmul(
                        ps, ones_sb[:], sq_qT[:, c * 512:(c + 1) * 512],
                        start=True, stop=True,
                    )
                    act(qs16[:, c * 512:(c + 1) * 512], ps, AF.Copy)

            # ---- main: GEMM2 + power into fp16 strips, one sqrt batch,
            # one exp batch (in-place), then GEMM3 per chunk ----
            with (
                tc.tile_pool(name="strips", bufs=NCHUNK) as strips,
                tc.tile_pool(name="psqk", bufs=3, space="PSUM") as psqk,
                tc.tile_pool(name="psk", bufs=1, space="PSUM") as psk,
            ):
                strip = []
                for c in range(NCHUNK):
                    st = strips.tile([P, NJT * NI], F16, tag="strip",
                                     name=f"strip{c}")
                    strip.append(st)
                # jt-outer so each GEMM2 stationary tile is loaded once and
                # reused for all 4 query chunks (128 -> 32 weight loads).
                for jt in range(NJT):
                    for c in range(NCHUNK):
                        ps2 = psqk.tile([P, NI], F32, tag="qk")
                        nc.tensor.matmul(
                            ps2, projTm2[:, jt * P:(jt + 1) * P],
                            qT_mm[:, c * NI:(c + 1) * NI],
                            start=True, stop=True,
                        )
                        # power = (-2qk) + ksq[j] + qsq[i], fused DVE op
                        nc.vector.affine_then_add(
                            strip[c][:, jt * NI:(jt + 1) * NI], ps2,
                            qs16[:, c * NI:(c + 1) * NI],
                            scale=1.0, bias=ksq[:, jt:jt + 1],
                        )
                # one sqrt batch over everything (in-place on the strips)
                NH = NQ // 2    # [128, 4096] sub-strips halve ACT op count
                for c in range(NCHUNK):
                    for h in range(NH):
                        seg = strip[c][:, h * (NJT * NI // NH):
                                       (h + 1) * (NJT * NI // NH)]
                        act(seg, seg, AF.Sqrt)
                # one exp batch + GEMM3 per chunk
                for c in range(NCHUNK):
                    ps3 = psk.tile([P, NI], F32, tag="kacc")
                    for h in range(NH):
                        seg = strip[c][:, h * (NJT * NI // NH):
                                       (h + 1) * (NJT * NI // NH)]
                        act(seg, seg, AF.Exp, scale=-1.0)
                        for j in range(NJT // NH):
                            jt = h * (NJT // NH) + j
                            nc.tensor.matmul(
                                ps3, projnat[:, jt * P:(jt + 1) * P],
                                strip[c][:, jt * NI:(jt + 1) * NI],
                                start=(jt == 0), stop=(jt == NJT - 1),
                            )
                    kT_tile = work.tile([P, NI], F32, tag="kout")
                    nc.vector.tensor_copy(kT_tile[:], ps3)
                    nc.sync.dma_start(
                        out=kT_out[:, c * NI:(c + 1) * NI], in_=kT_tile[:])

    nc.compile()
    return nc


_NC_CACHE = {}


def _get_nc():
    if "nc" not in _NC_CACHE:
        _NC_CACHE["nc"] = build_nc()
    return _NC_CACHE["nc"]


def kernel(x, W, b):
    x = np.asarray(x, dtype=np.float32)
    W = np.asarray(W, dtype=np.float32)
    b = np.asarray(b, dtype=np.float32)

    nc = _get_nc()

    WT = np.ascontiguousarray(W.T)
    Wm2T = np.ascontiguousarray((-2.0 * W).T)
    bcols = np.stack([b, -2.0 * b], axis=1).astype(np.float32)
    b_bcast = np.broadcast_to(b.reshape(1, D), (P, D)).astype(np.float32)
    b_bcast = np.ascontiguousarray(b_bcast)
    ones_mat = np.ones((P, P), np.float32)

    in_maps = []
    for i in range(B):
        in_maps.append({
            "xT": np.ascontiguousarray(x[i].T),
            "WT": WT,
            "Wm2T": Wm2T,
            "bcols": bcols,
            "b_bcast": b_bcast,
            "ones_mat": ones_mat,
        })

    trace = bool(int(os.environ.get("KBENCH_TRACE", "0")))
    kres = None
    last_exc = None
    for attempt in range(5):
        try:
            kres = run_bass_kernel_spmd(nc, in_maps, list(range(B)), trace=trace)
            break
        except Exception as exc:  # transient NRT_EXEC_UNIT_UNRECOVERABLE etc.
            last_exc = exc
            import time as _time
            _time.sleep(3.0 * (attempt + 1))
    if kres is None:
        raise last_exc
    _NC_CACHE["last_result"] = kres
    res = kres.results

    q = np.stack([np.ascontiguousarray(r["qT"].T) for r in res])
    k = np.stack([np.ascontiguousarray(r["kT"].T) for r in res])
    return q, k, k
